# revision 32
# baseline (speedup 1.0000x reference)
"""Trainium2 Bass kernel for batched tiny-projection attention.

Reference computation (per batch b):
    qp = relu(q @ W1.T + b1)            [Nq, 3]
    kp = relu(k @ W2.T + b2)            [Nf, 3]
    scores = (qp @ kp.T) / sqrt(3)      [Nq, Nf]
    attn = softmax(scores, axis=-1)
    out = attn @ v                      [Nq, C]

Shapes: B=4, Nq=2048, Nf=16384, D=3, C=768, fp32.

Sharding: 8 cores = (4 batches) x (2 halves of Nq). Each core handles
q[b, h*1024:(h+1)*1024], full k[b]/v[b], so softmax is local to a core
(no cross-core reduction needed).

Device algorithm (per core), oriented for the tensor engine:
  - The tiny projections (and their fp16 hi/lo splits) are computed on
    the HOST (Nf*9 flops — 0.001% of the work) and shipped as the
    [128, n] score-matmul operands directly: 4 partition blocks at
    {0,32,64,96} hold q:[hi,lo,hi,lo] x k:[hi,hi,lo,lo] so ONE K=128
    fp16 matmul per tile yields exact fp32-grade scores at 1 cycle/row
    (PE cost depends only on the moving free dim, not K).
  - kp rows that are all-zero after ReLU (~4.5%) score 0 against every
    query, i.e. a query-independent weight exp(-shift). They are
    compacted out on the host; their closed-form contribution
    exp(-shift)*[sum_z v | count_z] is added into the accumulator via
    the first group's PSUM flush (tensor_add with the vz tile).
  - scores are computed TRANSPOSED: sT[m, n] = kp[m]. qp[n], because the
    attn @ v matmul needs the contraction dim (m) on partitions.
  - exp(scale*s - shift) runs on the scalar engine straight out of PSUM,
    emitting bf16 tiles (bf16 range avoids underflow for rows whose max
    score is far below the global shift; scores >= 0 since qp,kp >= 0).
  - attn @ v accumulates in PSUM over a group of m-tiles, then is
    flushed (added) into an SBUF fp32 accumulator; v carries an extra
    ones column so the softmax denominator falls out of the same matmul
    (padding rows have ones-column 0 so they add nothing).
  - Final: out = acc[:, :768] * (1 / acc[:, 768]) per row, written fp16
    (5e-4 rel err, negligible vs the 2.3e-3 total) and emitted per-chunk
    inside the last group so the serial tail is short.
  - Startup: group 0's k/v DMA triggers ride the Activation HWDGE queue
    in parallel with Sync's qsplit/shift (each trigger costs ~600ns of
    serial sequencer time, which otherwise gates the first matmul).
"""

import sys

sys.path.insert(0, "/opt/trn_rl_repo")

import numpy as np

import concourse.bass as bass
import concourse.bacc as bacc
import concourse.tile as tile
from concourse import mybir
from concourse.bass_utils import run_bass_kernel_spmd

F32 = mybir.dt.float32
F16 = mybir.dt.float16
BF16 = mybir.dt.bfloat16

B, NQ_FULL, NF, D, C = 4, 2048, 16384, 3, 768
SCALE = 1.0 / np.sqrt(3.0)
NQ = NQ_FULL // 2          # per-core query rows
CA, CB = 512, C + 1 - 512  # c-chunk split of [v | ones] (769 = 512 + 257)
G = 16                     # max m-tiles (of 128) per group


def _group_sizes(m_tiles):
    """Ramp in with small groups so the first attn chunk only waits on a
    few exp tiles (PE would otherwise idle >3.4us and HAM re-throttles)."""
    sizes = []
    rem = m_tiles
    for s in (4, 4, 8):
        if rem <= 0:
            break
        s = min(s, rem)
        sizes.append(s)
        rem -= s
    while rem > 0:
        s = min(G, rem)
        sizes.append(s)
        rem -= s
    return sizes


def build_nc(m_tiles, nq=NQ, num_devices=8, warmup=0):
    """Build the single-core SPMD program for nf = m_tiles * 128
    (compacted+padded field rows)."""
    nf = m_tiles * 128
    assert nq % 512 == 0
    nchunks = nq // 128
    caug = C + 1
    sizes = _group_sizes(m_tiles)
    starts = [sum(sizes[:i]) for i in range(len(sizes))]
    assert len(sizes) >= 2

    nc = bacc.Bacc("TRN2", target_bir_lowering=False, debug=False,
                   num_devices=num_devices)

    qsplit_d = nc.dram_tensor("qsplit", [128, nq], F16, kind="ExternalInput")
    ksplit_d = nc.dram_tensor("ksplit", [128, nf], F16, kind="ExternalInput")
    # v pre-transposed on host: vaug[p, t, c] = v_row(t*128 + p), so each
    # group is ONE DMA trigger (the serial ~600ns/trigger SP time and the
    # per-DMA semaphores were the startup+epilogue bottleneck)
    vaug_d = nc.dram_tensor("vaug", [128, m_tiles, caug], BF16,
                            kind="ExternalInput")
    vz_d = nc.dram_tensor("vz", [128, caug], F32, kind="ExternalInput")
    shift_d = nc.dram_tensor("shift", [128, 1], F32, kind="ExternalInput")
    out_d = nc.dram_tensor("out", [nq, C], F16, kind="ExternalOutput")

    with tile.TileContext(nc) as tc, \
         tc.tile_pool(name="const", bufs=1) as const, \
         tc.tile_pool(name="kio", bufs=2) as kio, \
         tc.tile_pool(name="vp", bufs=2 * G) as vp, \
         tc.tile_pool(name="expp", bufs=2 * G) as expp, \
         tc.tile_pool(name="outp", bufs=3) as outp, \
         tc.tile_pool(name="recp", bufs=2) as recp, \
         tc.tile_pool(name="warm_ps", bufs=1, space="PSUM") as warm_ps, \
         tc.tile_pool(name="sc_ps", bufs=4, space="PSUM") as sc_ps, \
         tc.tile_pool(name="oA_ps", bufs=2, space="PSUM") as oA_ps, \
         tc.tile_pool(name="oB_ps", bufs=2, space="PSUM") as oB_ps:

        # ---- constants / once-per-core prologue ----
        qsplit_sb = const.tile([128, nq], F16)
        nc.sync.dma_start(qsplit_sb[:], qsplit_d[:])
        shift_sb = const.tile([128, 1], F32)
        nc.sync.dma_start(shift_sb[:], shift_d[:])

        # warmup matmuls on a zeroed scratch tile: keep PE busy while the
        # first DMAs land so the p-state ramps to full clock before real
        # work starts (PE idle >3us triggers a HAM re-throttle)
        if warmup:
            # scratch zeroed on the gpsimd engine (same memset the framework
            # itself uses for barrier buffers); PSUM result never read
            warm = const.tile([128, 128], F16)
            nc.gpsimd.memset(warm[:], 0.0)
            wps = warm_ps.tile([128, 128], F32)
            for _ in range(warmup):
                nc.tensor.matmul(wps[:], warm[:], warm[:], start=True,
                                 stop=True)

        acc = const.tile([128, nchunks, caug], F32)

        def emit_k(m0_tiles, size, eng=None):
            kt = kio.tile([128, G * 128], F16)
            c0 = m0_tiles * 128
            (eng or nc.sync).dma_start(kt[:, 0:size * 128],
                                       ksplit_d[:, c0:c0 + size * 128])
            return kt

        def emit_v(m0_tiles, size, eng=None):
            vts = []
            for t in range(size):
                vt = vp.tile([128, caug], BF16)
                (eng or nc.sync).dma_start(vt[:], vaug_d[:, m0_tiles + t, :])
                vts.append(vt)
            return vts

        def emit_scores(kt, ts, h_major=False):
            """scores + exp for m-tiles ts (local idx within group).
            h_major orders the low n-columns of every tile first, so the
            first attn chunk's dependencies complete earliest."""
            es = []
            for t in ts:
                et = expp.tile([128, nq], BF16, tag="e")
                es.append(et)
            ts = list(ts)
            order = [(h, j) for h in range(nq // 512) for j in range(len(ts))]
            if not h_major:
                order = [(h, j) for j in range(len(ts)) for h in range(nq // 512)]
            for h, j in order:
                t = ts[j]
                sp = sc_ps.tile([128, 512], F32)
                nc.tensor.matmul(sp[:], kt[:, t * 128:(t + 1) * 128],
                                 qsplit_sb[:, h * 512:(h + 1) * 512],
                                 start=True, stop=True)
                nc.scalar.activation(es[j][:, h * 512:(h + 1) * 512], sp[:],
                                     mybir.ActivationFunctionType.Exp,
                                     bias=shift_sb[:], scale=float(SCALE))
            return es

        def emit_scores_g0(kt, size):
            """Group-0 scores: h=0 exps in 128-col pieces, piece-major
            across tiles, so attn chunk 0's dependencies (col 0:128 of
            every tile) finish after `size` narrow exps instead of `size`
            full 512-wide ones. Needs `size` concurrent scores-PSUM bufs."""
            assert size <= 4
            es = [expp.tile([128, nq], BF16, tag="e", name="e0")
                  for _ in range(size)]
            sps = []
            for t in range(size):
                sp = sc_ps.tile([128, 512], F32)
                nc.tensor.matmul(sp[:], kt[:, t * 128:(t + 1) * 128],
                                 qsplit_sb[:, 0:512], start=True, stop=True)
                sps.append(sp)
            for p in range(4):
                for t in range(size):
                    nc.scalar.activation(
                        es[t][:, p * 128:(p + 1) * 128],
                        sps[t][:, p * 128:(p + 1) * 128],
                        mybir.ActivationFunctionType.Exp,
                        bias=shift_sb[:], scale=float(SCALE))
            for t in range(size):
                sp = sc_ps.tile([128, 512], F32)
                nc.tensor.matmul(sp[:], kt[:, t * 128:(t + 1) * 128],
                                 qsplit_sb[:, 512:1024], start=True, stop=True)
                nc.scalar.activation(es[t][:, 512:1024], sp[:],
                                     mybir.ActivationFunctionType.Exp,
                                     bias=shift_sb[:], scale=float(SCALE))
            return es

        def emit_attn_chunk(first_group, last_group, ci, size, es, vts):
            pA = oA_ps.tile([128, CA], F32)
            pB = oB_ps.tile([128, CB], F32)
            for i in range(size):
                e = es[i][:, ci * 128:(ci + 1) * 128]
                mms = [(pA, 0, CA), (pB, CA, caug)]
                if last_group:
                    # B (denominator) last-matmul finishes first, so its
                    # flush + the reciprocal overlap the final A matmul
                    mms.reverse()
                for pt, c0, c1 in mms:
                    nc.tensor.matmul(pt[:], e, vts[i][:, c0:c1],
                                     start=(i == 0), stop=(i == size - 1))
            # flush B (with the denominator) first so the last chunk's
            # reciprocal overlaps the A flush
            if first_group:
                # fold in the host-computed zero-row contribution
                nc.vector.tensor_add(acc[:, ci, CA:caug], vz_sb[:, CA:caug],
                                     pB[:])
            else:
                nc.vector.tensor_add(acc[:, ci, CA:caug], acc[:, ci, CA:caug],
                                     pB[:])
            if last_group:
                rec = recp.tile([128, 1], F32)
                nc.vector.reciprocal(rec[:], acc[:, ci, C:caug])
            if first_group:
                nc.vector.tensor_add(acc[:, ci, 0:CA], vz_sb[:, 0:CA], pA[:])
            else:
                nc.vector.tensor_add(acc[:, ci, 0:CA], acc[:, ci, 0:CA], pA[:])
            if last_group:
                # normalize and store this chunk immediately (short tail)
                ot = outp.tile([128, C], F16)
                nc.vector.tensor_scalar_mul(ot[:], acc[:, ci, 0:C], rec[:])
                nc.sync.dma_start(out_d[ci * 128:(ci + 1) * 128, :], ot[:])

        # ---- software-pipelined main loop ----
        # group 0's k/v DMAs go out on the Activation HWDGE queue so their
        # ~600ns triggers issue in parallel with Sync's qsplit/shift (the
        # serial trigger chain otherwise gates the first matmul)
        ks_cur = emit_k(starts[0], sizes[0], eng=nc.scalar)
        # group-0 v tiles alternate Sync/Activation queues so all four land
        # ~1us earlier than a single serial trigger chain
        v_cur = []
        for t in range(sizes[0]):
            vt = vp.tile([128, caug], BF16)
            eng = nc.sync if t % 2 == 0 else nc.scalar
            eng.dma_start(vt[:], vaug_d[:, starts[0] + t, :])
            v_cur.append(vt)
        vz_sb = const.tile([128, caug], F32)
        nc.scalar.dma_start(vz_sb[:], vz_d[:])
        if sizes[0] <= 4 and nq == 1024:
            e_cur = emit_scores_g0(ks_cur, sizes[0])
        else:
            e_cur = emit_scores(ks_cur, range(sizes[0]), h_major=True)
        for gi in range(len(sizes)):
            last = gi + 1 >= len(sizes)
            if not last:
                ks_nxt = emit_k(starts[gi + 1], sizes[gi + 1])
                v_nxt = emit_v(starts[gi + 1], sizes[gi + 1])
                e_nxt = []
            # distribute next group's score matmuls across this group's
            # attn chunks to keep PE dense and ACT fed early
            for ci in range(nchunks):
                emit_attn_chunk(gi == 0, last, ci, sizes[gi], e_cur, v_cur)
                if not last:
                    nnx = sizes[gi + 1]
                    per = (nnx + nchunks - 1) // nchunks
                    ts = range(ci * per, min((ci + 1) * per, nnx))
                    e_nxt.extend(emit_scores(ks_nxt, ts))
            if not last:
                ks_cur, v_cur, e_cur = ks_nxt, v_nxt, e_nxt

    nc.finalize()
    return nc


def _split16(x):
    hi = x.astype(np.float16)
    lo = (x - hi.astype(np.float32)).astype(np.float16)
    return hi, lo


def _qsplit(qp):
    """[n, 3] projected queries -> [128, n] fp16, blocks [hi, lo, hi, lo]."""
    hi, lo = _split16(np.ascontiguousarray(qp.T.astype(np.float32)))
    m = np.zeros((128, qp.shape[0]), np.float16)
    for e in range(3):
        m[0 + e] = hi[e]
        m[32 + e] = lo[e]
        m[64 + e] = hi[e]
        m[96 + e] = lo[e]
    return m


def _ksplit(kp, nf_eff):
    """[nz, 3] projected keys -> [128, nf_eff] fp16, blocks
    [hi, hi, lo, lo], zero-padded beyond nz."""
    hi, lo = _split16(np.ascontiguousarray(kp.T.astype(np.float32)))
    m = np.zeros((128, nf_eff), np.float16)
    nz = kp.shape[0]
    for e in range(3):
        m[0 + e, :nz] = hi[e]
        m[32 + e, :nz] = hi[e]
        m[64 + e, :nz] = lo[e]
        m[96 + e, :nz] = lo[e]
    return m


def _host_prep(q, k, v, W1, b1, W2, b2):
    """Per-core input maps: tiny projections, fp16 splits, zero-row
    compaction, shift bound. O(N*D) host work only."""
    import ml_dtypes

    caug = C + 1
    qps, kps, nzs = [], [], []
    for b in range(B):
        qps.append(np.maximum(q[b].astype(np.float32) @ W1.T.astype(np.float32)
                              + b1.astype(np.float32), 0.0))
        kps.append(np.maximum(k[b].astype(np.float32) @ W2.T.astype(np.float32)
                              + b2.astype(np.float32), 0.0))
        nzs.append(np.flatnonzero((kps[b] > 0.0).any(axis=1)))
    nf_eff = max((len(nz) + 127) // 128 * 128 for nz in nzs)

    per_batch = {}
    for b in range(B):
        qp, kp, nz = qps[b], kps[b], nzs[b]
        nzmask = np.zeros(NF, dtype=bool)
        nzmask[nz] = True
        zcount = NF - len(nz)
        # cheap per-batch upper bound on max score -> exp(s - shift) <= 1
        bound = SCALE * float(qp.max(axis=0) @ kp.max(axis=0))
        w0 = float(np.exp(-bound))

        va = np.zeros((nf_eff, caug), np.float32)
        va[:len(nz), :C] = v[b][nz]
        va[:len(nz), C] = 1.0
        # [nf_eff, caug] -> [128, m_tiles, caug] with row(t*128+p) at [p, t]
        va = np.ascontiguousarray(
            va.reshape(nf_eff // 128, 128, caug).transpose(1, 0, 2))

        vzrow = np.empty(caug, np.float64)
        vzrow[:C] = v[b][~nzmask].astype(np.float64).sum(axis=0) * w0
        vzrow[C] = zcount * w0

        per_batch[b] = {
            "ksplit": _ksplit(kp[nz], nf_eff),
            "vaug": va.astype(ml_dtypes.bfloat16),
            "vz": np.ascontiguousarray(
                np.tile(vzrow.astype(np.float32), (128, 1))),
            "shift": np.full((128, 1), -bound, np.float32),
        }

    in_maps = []
    for core in range(8):
        b, h = core // 2, core % 2
        in_maps.append({
            "qsplit": _qsplit(qps[b][h * NQ:(h + 1) * NQ]),
            **per_batch[b],
        })
    return in_maps, nf_eff


_NC_CACHE = {}


def kernel(q, k, v, W1, b1, W2, b2, _trace=False):
    q, k, v = np.asarray(q), np.asarray(k), np.asarray(v)
    W1, b1 = np.asarray(W1), np.asarray(b1)
    W2, b2 = np.asarray(W2), np.asarray(b2)

    in_maps, nf_eff = _host_prep(q, k, v, W1, b1, W2, b2)
    m_tiles = nf_eff // 128
    if m_tiles not in _NC_CACHE:
        _NC_CACHE[m_tiles] = build_nc(m_tiles)
    nc = _NC_CACHE[m_tiles]

    res = run_bass_kernel_spmd(nc, in_maps, list(range(8)), trace=_trace)

    out = np.empty((B, NQ_FULL, C), np.float32)
    for core in range(8):
        b, h = core // 2, core % 2
        out[b, h * NQ:(h + 1) * NQ, :] = np.asarray(
            res.results[core]["out"]).astype(np.float32)
    if _trace:
        return out, res
    return out


# revision 33
# speedup vs baseline: 1.0046x; 1.0046x over previous
"""Trainium2 Bass kernel for batched tiny-projection attention.

Reference computation (per batch b):
    qp = relu(q @ W1.T + b1)            [Nq, 3]
    kp = relu(k @ W2.T + b2)            [Nf, 3]
    scores = (qp @ kp.T) / sqrt(3)      [Nq, Nf]
    attn = softmax(scores, axis=-1)
    out = attn @ v                      [Nq, C]

Shapes: B=4, Nq=2048, Nf=16384, D=3, C=768, fp32.

Sharding: 8 cores = (4 batches) x (2 halves of Nq). Each core handles
q[b, h*1024:(h+1)*1024], full k[b]/v[b], so softmax is local to a core
(no cross-core reduction needed).

Device algorithm (per core), oriented for the tensor engine:
  - The tiny projections (and their fp16 hi/lo splits) are computed on
    the HOST (Nf*9 flops — 0.001% of the work) and shipped as the
    [128, n] score-matmul operands directly: 4 partition blocks at
    {0,32,64,96} hold q:[hi,lo,hi,lo] x k:[hi,hi,lo,lo] so ONE K=128
    fp16 matmul per tile yields exact fp32-grade scores at 1 cycle/row
    (PE cost depends only on the moving free dim, not K).
  - kp rows that are all-zero after ReLU (~4.5%) score 0 against every
    query, i.e. a query-independent weight exp(-shift). They are
    compacted out on the host; their closed-form contribution
    exp(-shift)*[sum_z v | count_z] is added into the accumulator via
    the first group's PSUM flush (tensor_add with the vz tile).
  - scores are computed TRANSPOSED: sT[m, n] = kp[m]. qp[n], because the
    attn @ v matmul needs the contraction dim (m) on partitions.
  - exp(scale*s - shift) runs on the scalar engine straight out of PSUM,
    emitting bf16 tiles (bf16 range avoids underflow for rows whose max
    score is far below the global shift; scores >= 0 since qp,kp >= 0).
  - attn @ v accumulates in PSUM over a group of m-tiles, then is
    flushed (added) into an SBUF fp32 accumulator; v carries an extra
    ones column so the softmax denominator falls out of the same matmul
    (padding rows have ones-column 0 so they add nothing).
  - Final: out = acc[:, :768] * (1 / acc[:, 768]) per row, written fp16
    (5e-4 rel err, negligible vs the 2.3e-3 total) and emitted per-chunk
    inside the last group so the serial tail is short.
  - Startup: group 0's k/v DMA triggers ride the Activation HWDGE queue
    in parallel with Sync's qsplit/shift (each trigger costs ~600ns of
    serial sequencer time, which otherwise gates the first matmul).
"""

import sys

sys.path.insert(0, "/opt/trn_rl_repo")

import numpy as np

import concourse.bass as bass
import concourse.bacc as bacc
import concourse.tile as tile
from concourse import mybir
from concourse.bass_utils import run_bass_kernel_spmd

F32 = mybir.dt.float32
F16 = mybir.dt.float16
BF16 = mybir.dt.bfloat16

B, NQ_FULL, NF, D, C = 4, 2048, 16384, 3, 768
SCALE = 1.0 / np.sqrt(3.0)
NQ = NQ_FULL // 2          # per-core query rows
CA, CB = 512, C + 1 - 512  # c-chunk split of [v | ones] (769 = 512 + 257)
G = 16                     # max m-tiles (of 128) per group


def _group_sizes(m_tiles):
    """Ramp in with small groups so the first attn chunk only waits on a
    few exp tiles (PE would otherwise idle >3.4us and HAM re-throttles)."""
    sizes = []
    rem = m_tiles
    for s in (4, 4, 8):
        if rem <= 0:
            break
        s = min(s, rem)
        sizes.append(s)
        rem -= s
    while rem > 0:
        s = min(G, rem)
        sizes.append(s)
        rem -= s
    return sizes


def build_nc(m_tiles, nq=NQ, num_devices=8, warmup=0):
    """Build the single-core SPMD program for nf = m_tiles * 128
    (compacted+padded field rows)."""
    nf = m_tiles * 128
    assert nq % 512 == 0
    nchunks = nq // 128
    caug = C + 1
    sizes = _group_sizes(m_tiles)
    starts = [sum(sizes[:i]) for i in range(len(sizes))]
    assert len(sizes) >= 2

    nc = bacc.Bacc("TRN2", target_bir_lowering=False, debug=False,
                   num_devices=num_devices)

    qsplit_d = nc.dram_tensor("qsplit", [128, nq], F16, kind="ExternalInput")
    ksplit_d = nc.dram_tensor("ksplit", [128, nf], F16, kind="ExternalInput")
    # v pre-transposed on host: vaug[p, t, c] = v_row(t*128 + p), so each
    # group is ONE DMA trigger (the serial ~600ns/trigger SP time and the
    # per-DMA semaphores were the startup+epilogue bottleneck)
    vaug_d = nc.dram_tensor("vaug", [128, m_tiles, caug], BF16,
                            kind="ExternalInput")
    vz_d = nc.dram_tensor("vz", [128, caug], F32, kind="ExternalInput")
    shift_d = nc.dram_tensor("shift", [128, 1], F32, kind="ExternalInput")
    out_d = nc.dram_tensor("out", [nq, C], F16, kind="ExternalOutput")

    with tile.TileContext(nc) as tc, \
         tc.tile_pool(name="const", bufs=1) as const, \
         tc.tile_pool(name="kio", bufs=2) as kio, \
         tc.tile_pool(name="vp", bufs=2 * G) as vp, \
         tc.tile_pool(name="expp", bufs=2 * G) as expp, \
         tc.tile_pool(name="outp", bufs=3) as outp, \
         tc.tile_pool(name="recp", bufs=2) as recp, \
         tc.tile_pool(name="warm_ps", bufs=1, space="PSUM") as warm_ps, \
         tc.tile_pool(name="sc_ps", bufs=3, space="PSUM") as sc_ps, \
         tc.tile_pool(name="oA_ps", bufs=2, space="PSUM") as oA_ps, \
         tc.tile_pool(name="oB_ps", bufs=2, space="PSUM") as oB_ps:

        # ---- constants / once-per-core prologue ----
        qsplit_sb = const.tile([128, nq], F16)
        nc.sync.dma_start(qsplit_sb[:], qsplit_d[:])
        shift_sb = const.tile([128, 1], F32)
        nc.sync.dma_start(shift_sb[:], shift_d[:])

        # warmup matmuls on a zeroed scratch tile: keep PE busy while the
        # first DMAs land so the p-state ramps to full clock before real
        # work starts (PE idle >3us triggers a HAM re-throttle)
        if warmup:
            # scratch zeroed on the gpsimd engine (same memset the framework
            # itself uses for barrier buffers); PSUM result never read
            warm = const.tile([128, 128], F16)
            nc.gpsimd.memset(warm[:], 0.0)
            wps = warm_ps.tile([128, 128], F32)
            for _ in range(warmup):
                nc.tensor.matmul(wps[:], warm[:], warm[:], start=True,
                                 stop=True)

        acc = const.tile([128, nchunks, caug], F32)

        def emit_k(m0_tiles, size, eng=None):
            kt = kio.tile([128, G * 128], F16)
            c0 = m0_tiles * 128
            (eng or nc.sync).dma_start(kt[:, 0:size * 128],
                                       ksplit_d[:, c0:c0 + size * 128])
            return kt

        def emit_v(m0_tiles, size, eng=None):
            vts = []
            for t in range(size):
                vt = vp.tile([128, caug], BF16)
                (eng or nc.sync).dma_start(vt[:], vaug_d[:, m0_tiles + t, :])
                vts.append(vt)
            return vts

        def emit_scores(kt, ts, h_major=False):
            """scores + exp for m-tiles ts (local idx within group).
            h_major orders the low n-columns of every tile first, so the
            first attn chunk's dependencies complete earliest."""
            es = []
            for t in ts:
                et = expp.tile([128, nq], BF16, tag="e")
                es.append(et)
            ts = list(ts)
            order = [(h, j) for h in range(nq // 512) for j in range(len(ts))]
            if not h_major:
                order = [(h, j) for j in range(len(ts)) for h in range(nq // 512)]
            for h, j in order:
                t = ts[j]
                sp = sc_ps.tile([128, 512], F32)
                nc.tensor.matmul(sp[:], kt[:, t * 128:(t + 1) * 128],
                                 qsplit_sb[:, h * 512:(h + 1) * 512],
                                 start=True, stop=True)
                nc.scalar.activation(es[j][:, h * 512:(h + 1) * 512], sp[:],
                                     mybir.ActivationFunctionType.Exp,
                                     bias=shift_sb[:], scale=float(SCALE))
            return es

        def emit_scores_g0(kt, size):
            """Group-0 scores: h=0 exps in 128-col pieces, piece-major
            across tiles, so attn chunk 0's dependencies (col 0:128 of
            every tile) finish after `size` narrow exps instead of `size`
            full 512-wide ones. Needs `size` concurrent scores-PSUM bufs."""
            assert size <= 4
            es = [expp.tile([128, nq], BF16, tag="e", name="e0")
                  for _ in range(size)]
            sps = []
            for t in range(size):
                sp = sc_ps.tile([128, 512], F32)
                nc.tensor.matmul(sp[:], kt[:, t * 128:(t + 1) * 128],
                                 qsplit_sb[:, 0:512], start=True, stop=True)
                sps.append(sp)
            for p in range(4):
                for t in range(size):
                    nc.scalar.activation(
                        es[t][:, p * 128:(p + 1) * 128],
                        sps[t][:, p * 128:(p + 1) * 128],
                        mybir.ActivationFunctionType.Exp,
                        bias=shift_sb[:], scale=float(SCALE))
            for t in range(size):
                sp = sc_ps.tile([128, 512], F32)
                nc.tensor.matmul(sp[:], kt[:, t * 128:(t + 1) * 128],
                                 qsplit_sb[:, 512:1024], start=True, stop=True)
                nc.scalar.activation(es[t][:, 512:1024], sp[:],
                                     mybir.ActivationFunctionType.Exp,
                                     bias=shift_sb[:], scale=float(SCALE))
            return es

        def emit_attn_chunk(first_group, last_group, ci, size, es, vts):
            pA = oA_ps.tile([128, CA], F32)
            pB = oB_ps.tile([128, CB], F32)
            for i in range(size):
                e = es[i][:, ci * 128:(ci + 1) * 128]
                mms = [(pA, 0, CA), (pB, CA, caug)]
                if last_group:
                    # B (denominator) last-matmul finishes first, so its
                    # flush + the reciprocal overlap the final A matmul
                    mms.reverse()
                for pt, c0, c1 in mms:
                    nc.tensor.matmul(pt[:], e, vts[i][:, c0:c1],
                                     start=(i == 0), stop=(i == size - 1))
            # flush B (with the denominator) first so the last chunk's
            # reciprocal overlaps the A flush
            if first_group:
                # fold in the host-computed zero-row contribution
                nc.vector.tensor_add(acc[:, ci, CA:caug], vz_sb[:, CA:caug],
                                     pB[:])
            else:
                nc.vector.tensor_add(acc[:, ci, CA:caug], acc[:, ci, CA:caug],
                                     pB[:])
            if last_group:
                rec = recp.tile([128, 1], F32)
                nc.vector.reciprocal(rec[:], acc[:, ci, C:caug])
            if first_group:
                nc.vector.tensor_add(acc[:, ci, 0:CA], vz_sb[:, 0:CA], pA[:])
            else:
                nc.vector.tensor_add(acc[:, ci, 0:CA], acc[:, ci, 0:CA], pA[:])
            if last_group:
                # normalize and store this chunk immediately (short tail)
                ot = outp.tile([128, C], F16)
                nc.vector.tensor_scalar_mul(ot[:], acc[:, ci, 0:C], rec[:])
                nc.sync.dma_start(out_d[ci * 128:(ci + 1) * 128, :], ot[:])

        # ---- software-pipelined main loop ----
        # group 0's k/v DMAs go out on the Activation HWDGE queue so their
        # ~600ns triggers issue in parallel with Sync's qsplit/shift (the
        # serial trigger chain otherwise gates the first matmul)
        ks_cur = emit_k(starts[0], sizes[0], eng=nc.scalar)
        v_cur = emit_v(starts[0], sizes[0], eng=nc.scalar)
        vz_sb = const.tile([128, caug], F32)
        nc.scalar.dma_start(vz_sb[:], vz_d[:])
        e_cur = emit_scores(ks_cur, range(sizes[0]), h_major=True)
        for gi in range(len(sizes)):
            last = gi + 1 >= len(sizes)
            if not last:
                ks_nxt = emit_k(starts[gi + 1], sizes[gi + 1])
                v_nxt = emit_v(starts[gi + 1], sizes[gi + 1])
                e_nxt = []
            # distribute next group's score matmuls across this group's
            # attn chunks to keep PE dense and ACT fed early
            for ci in range(nchunks):
                emit_attn_chunk(gi == 0, last, ci, sizes[gi], e_cur, v_cur)
                if not last:
                    nnx = sizes[gi + 1]
                    per = (nnx + nchunks - 1) // nchunks
                    ts = range(ci * per, min((ci + 1) * per, nnx))
                    e_nxt.extend(emit_scores(ks_nxt, ts))
            if not last:
                ks_cur, v_cur, e_cur = ks_nxt, v_nxt, e_nxt

    nc.finalize()
    return nc


def _split16(x):
    hi = x.astype(np.float16)
    lo = (x - hi.astype(np.float32)).astype(np.float16)
    return hi, lo


def _qsplit(qp):
    """[n, 3] projected queries -> [128, n] fp16, blocks [hi, lo, hi, lo]."""
    hi, lo = _split16(np.ascontiguousarray(qp.T.astype(np.float32)))
    m = np.zeros((128, qp.shape[0]), np.float16)
    for e in range(3):
        m[0 + e] = hi[e]
        m[32 + e] = lo[e]
        m[64 + e] = hi[e]
        m[96 + e] = lo[e]
    return m


def _ksplit(kp, nf_eff):
    """[nz, 3] projected keys -> [128, nf_eff] fp16, blocks
    [hi, hi, lo, lo], zero-padded beyond nz."""
    hi, lo = _split16(np.ascontiguousarray(kp.T.astype(np.float32)))
    m = np.zeros((128, nf_eff), np.float16)
    nz = kp.shape[0]
    for e in range(3):
        m[0 + e, :nz] = hi[e]
        m[32 + e, :nz] = hi[e]
        m[64 + e, :nz] = lo[e]
        m[96 + e, :nz] = lo[e]
    return m


def _host_prep(q, k, v, W1, b1, W2, b2):
    """Per-core input maps: tiny projections, fp16 splits, zero-row
    compaction, shift bound. O(N*D) host work only."""
    import ml_dtypes

    caug = C + 1
    qps, kps, nzs = [], [], []
    for b in range(B):
        qps.append(np.maximum(q[b].astype(np.float32) @ W1.T.astype(np.float32)
                              + b1.astype(np.float32), 0.0))
        kps.append(np.maximum(k[b].astype(np.float32) @ W2.T.astype(np.float32)
                              + b2.astype(np.float32), 0.0))
        nzs.append(np.flatnonzero((kps[b] > 0.0).any(axis=1)))
    nf_eff = max((len(nz) + 127) // 128 * 128 for nz in nzs)

    per_batch = {}
    for b in range(B):
        qp, kp, nz = qps[b], kps[b], nzs[b]
        nzmask = np.zeros(NF, dtype=bool)
        nzmask[nz] = True
        zcount = NF - len(nz)
        # cheap per-batch upper bound on max score -> exp(s - shift) <= 1
        bound = SCALE * float(qp.max(axis=0) @ kp.max(axis=0))
        w0 = float(np.exp(-bound))

        va = np.zeros((nf_eff, caug), np.float32)
        va[:len(nz), :C] = v[b][nz]
        va[:len(nz), C] = 1.0
        # [nf_eff, caug] -> [128, m_tiles, caug] with row(t*128+p) at [p, t]
        va = np.ascontiguousarray(
            va.reshape(nf_eff // 128, 128, caug).transpose(1, 0, 2))

        vzrow = np.empty(caug, np.float64)
        vzrow[:C] = v[b][~nzmask].astype(np.float64).sum(axis=0) * w0
        vzrow[C] = zcount * w0

        per_batch[b] = {
            "ksplit": _ksplit(kp[nz], nf_eff),
            "vaug": va.astype(ml_dtypes.bfloat16),
            "vz": np.ascontiguousarray(
                np.tile(vzrow.astype(np.float32), (128, 1))),
            "shift": np.full((128, 1), -bound, np.float32),
        }

    in_maps = []
    for core in range(8):
        b, h = core // 2, core % 2
        in_maps.append({
            "qsplit": _qsplit(qps[b][h * NQ:(h + 1) * NQ]),
            **per_batch[b],
        })
    return in_maps, nf_eff


_NC_CACHE = {}


def kernel(q, k, v, W1, b1, W2, b2, _trace=False):
    q, k, v = np.asarray(q), np.asarray(k), np.asarray(v)
    W1, b1 = np.asarray(W1), np.asarray(b1)
    W2, b2 = np.asarray(W2), np.asarray(b2)

    in_maps, nf_eff = _host_prep(q, k, v, W1, b1, W2, b2)
    m_tiles = nf_eff // 128
    if m_tiles not in _NC_CACHE:
        _NC_CACHE[m_tiles] = build_nc(m_tiles)
    nc = _NC_CACHE[m_tiles]

    res = run_bass_kernel_spmd(nc, in_maps, list(range(8)), trace=_trace)

    out = np.empty((B, NQ_FULL, C), np.float32)
    for core in range(8):
        b, h = core // 2, core % 2
        out[b, h * NQ:(h + 1) * NQ, :] = np.asarray(
            res.results[core]["out"]).astype(np.float32)
    if _trace:
        return out, res
    return out


# revision 34
# speedup vs baseline: 1.0060x; 1.0014x over previous
"""Trainium2 Bass kernel for batched tiny-projection attention.

Reference computation (per batch b):
    qp = relu(q @ W1.T + b1)            [Nq, 3]
    kp = relu(k @ W2.T + b2)            [Nf, 3]
    scores = (qp @ kp.T) / sqrt(3)      [Nq, Nf]
    attn = softmax(scores, axis=-1)
    out = attn @ v                      [Nq, C]

Shapes: B=4, Nq=2048, Nf=16384, D=3, C=768, fp32.

Sharding: 8 cores = (4 batches) x (2 halves of Nq). Each core handles
q[b, h*1024:(h+1)*1024], full k[b]/v[b], so softmax is local to a core
(no cross-core reduction needed).

Device algorithm (per core), oriented for the tensor engine:
  - The tiny projections (and their fp16 hi/lo splits) are computed on
    the HOST (Nf*9 flops — 0.001% of the work) and shipped as the
    [128, n] score-matmul operands directly: 4 partition blocks at
    {0,32,64,96} hold q:[hi,lo,hi,lo] x k:[hi,hi,lo,lo] so ONE K=128
    fp16 matmul per tile yields exact fp32-grade scores at 1 cycle/row
    (PE cost depends only on the moving free dim, not K).
  - kp rows that are all-zero after ReLU (~4.5%) score 0 against every
    query, i.e. a query-independent weight exp(-shift). They are
    compacted out on the host; their closed-form contribution
    exp(-shift)*[sum_z v | count_z] is added into the accumulator via
    the first group's PSUM flush (tensor_add with the vz tile).
  - scores are computed TRANSPOSED: sT[m, n] = kp[m]. qp[n], because the
    attn @ v matmul needs the contraction dim (m) on partitions.
  - exp(scale*s - shift) runs on the scalar engine straight out of PSUM,
    emitting bf16 tiles (bf16 range avoids underflow for rows whose max
    score is far below the global shift; scores >= 0 since qp,kp >= 0).
  - attn @ v accumulates in PSUM over a group of m-tiles, then is
    flushed (added) into an SBUF fp32 accumulator; v carries an extra
    ones column so the softmax denominator falls out of the same matmul
    (padding rows have ones-column 0 so they add nothing).
  - Final: out = acc[:, :768] * (1 / acc[:, 768]) per row, written fp16
    (5e-4 rel err, negligible vs the 2.3e-3 total) and emitted per-chunk
    inside the last group so the serial tail is short.
  - Startup: group 0's k/v DMA triggers ride the Activation HWDGE queue
    in parallel with Sync's qsplit/shift (each trigger costs ~600ns of
    serial sequencer time, which otherwise gates the first matmul).
"""

import sys

sys.path.insert(0, "/opt/trn_rl_repo")

import numpy as np

import concourse.bass as bass
import concourse.bacc as bacc
import concourse.tile as tile
from concourse import mybir
from concourse.bass_utils import run_bass_kernel_spmd

F32 = mybir.dt.float32
F16 = mybir.dt.float16
BF16 = mybir.dt.bfloat16

B, NQ_FULL, NF, D, C = 4, 2048, 16384, 3, 768
SCALE = 1.0 / np.sqrt(3.0)
NQ = NQ_FULL // 2          # per-core query rows
CA, CB = 512, C + 1 - 512  # c-chunk split of [v | ones] (769 = 512 + 257)
G = 16                     # max m-tiles (of 128) per group


def _group_sizes(m_tiles):
    """Ramp in with small groups so the first attn chunk only waits on a
    few exp tiles (PE would otherwise idle >3.4us and HAM re-throttles)."""
    sizes = []
    rem = m_tiles
    for s in (4, 4, 8):
        if rem <= 0:
            break
        s = min(s, rem)
        sizes.append(s)
        rem -= s
    while rem > 0:
        s = min(G, rem)
        sizes.append(s)
        rem -= s
    return sizes


def build_nc(m_tiles, nq=NQ, num_devices=8, warmup=0):
    """Build the single-core SPMD program for nf = m_tiles * 128
    (compacted+padded field rows)."""
    nf = m_tiles * 128
    assert nq % 512 == 0
    nchunks = nq // 128
    caug = C + 1
    sizes = _group_sizes(m_tiles)
    starts = [sum(sizes[:i]) for i in range(len(sizes))]
    assert len(sizes) >= 2

    nc = bacc.Bacc("TRN2", target_bir_lowering=False, debug=False,
                   num_devices=num_devices)

    qsplit_d = nc.dram_tensor("qsplit", [128, nq], F16, kind="ExternalInput")
    ksplit_d = nc.dram_tensor("ksplit", [128, nf], F16, kind="ExternalInput")
    # v pre-transposed on host: vaug[p, t, c] = v_row(t*128 + p), so each
    # group is ONE DMA trigger (the serial ~600ns/trigger SP time and the
    # per-DMA semaphores were the startup+epilogue bottleneck)
    vaug_d = nc.dram_tensor("vaug", [128, m_tiles, caug], BF16,
                            kind="ExternalInput")
    vz_d = nc.dram_tensor("vz", [128, caug], F32, kind="ExternalInput")
    shift_d = nc.dram_tensor("shift", [128, 1], F32, kind="ExternalInput")
    out_d = nc.dram_tensor("out", [nq, C], F16, kind="ExternalOutput")

    with tile.TileContext(nc) as tc, \
         tc.tile_pool(name="const", bufs=1) as const, \
         tc.tile_pool(name="kio", bufs=2) as kio, \
         tc.tile_pool(name="vp", bufs=2 * G) as vp, \
         tc.tile_pool(name="expp", bufs=2 * G) as expp, \
         tc.tile_pool(name="outp", bufs=3) as outp, \
         tc.tile_pool(name="recp", bufs=2) as recp, \
         tc.tile_pool(name="warm_ps", bufs=1, space="PSUM") as warm_ps, \
         tc.tile_pool(name="sc_ps", bufs=3, space="PSUM") as sc_ps, \
         tc.tile_pool(name="oA_ps", bufs=2, space="PSUM") as oA_ps, \
         tc.tile_pool(name="oB_ps", bufs=2, space="PSUM") as oB_ps:

        # ---- constants / once-per-core prologue ----
        qsplit_sb = const.tile([128, nq], F16)
        nc.sync.dma_start(qsplit_sb[:], qsplit_d[:])
        shift_sb = const.tile([128, 1], F32)
        nc.sync.dma_start(shift_sb[:], shift_d[:])

        # warmup matmuls on a zeroed scratch tile: keep PE busy while the
        # first DMAs land so the p-state ramps to full clock before real
        # work starts (PE idle >3us triggers a HAM re-throttle)
        if warmup:
            # scratch zeroed on the gpsimd engine (same memset the framework
            # itself uses for barrier buffers); PSUM result never read
            warm = const.tile([128, 128], F16)
            nc.gpsimd.memset(warm[:], 0.0)
            wps = warm_ps.tile([128, 128], F32)
            for _ in range(warmup):
                nc.tensor.matmul(wps[:], warm[:], warm[:], start=True,
                                 stop=True)

        acc = const.tile([128, nchunks, caug], F32)

        def emit_k(m0_tiles, size, eng=None):
            kt = kio.tile([128, G * 128], F16)
            c0 = m0_tiles * 128
            (eng or nc.sync).dma_start(kt[:, 0:size * 128],
                                       ksplit_d[:, c0:c0 + size * 128])
            return kt

        def emit_v(m0_tiles, size, eng=None):
            vts = []
            for t in range(size):
                vt = vp.tile([128, caug], BF16)
                (eng or nc.sync).dma_start(vt[:], vaug_d[:, m0_tiles + t, :])
                vts.append(vt)
            return vts

        def emit_scores(kt, ts, h_major=False):
            """scores + exp for m-tiles ts (local idx within group).
            h_major orders the low n-columns of every tile first, so the
            first attn chunk's dependencies complete earliest."""
            es = []
            for t in ts:
                et = expp.tile([128, nq], BF16, tag="e")
                es.append(et)
            ts = list(ts)
            order = [(h, j) for h in range(nq // 512) for j in range(len(ts))]
            if not h_major:
                order = [(h, j) for j in range(len(ts)) for h in range(nq // 512)]
            for h, j in order:
                t = ts[j]
                sp = sc_ps.tile([128, 512], F32)
                nc.tensor.matmul(sp[:], kt[:, t * 128:(t + 1) * 128],
                                 qsplit_sb[:, h * 512:(h + 1) * 512],
                                 start=True, stop=True)
                nc.scalar.activation(es[j][:, h * 512:(h + 1) * 512], sp[:],
                                     mybir.ActivationFunctionType.Exp,
                                     bias=shift_sb[:], scale=float(SCALE))
            return es

        def emit_scores_g0(kt, size):
            """Group-0 scores: h=0 exps in 128-col pieces, piece-major
            across tiles, so attn chunk 0's dependencies (col 0:128 of
            every tile) finish after `size` narrow exps instead of `size`
            full 512-wide ones. Needs `size` concurrent scores-PSUM bufs."""
            assert size <= 4
            es = [expp.tile([128, nq], BF16, tag="e", name="e0")
                  for _ in range(size)]
            sps = []
            for t in range(size):
                sp = sc_ps.tile([128, 512], F32)
                nc.tensor.matmul(sp[:], kt[:, t * 128:(t + 1) * 128],
                                 qsplit_sb[:, 0:512], start=True, stop=True)
                sps.append(sp)
            for p in range(4):
                for t in range(size):
                    nc.scalar.activation(
                        es[t][:, p * 128:(p + 1) * 128],
                        sps[t][:, p * 128:(p + 1) * 128],
                        mybir.ActivationFunctionType.Exp,
                        bias=shift_sb[:], scale=float(SCALE))
            for t in range(size):
                sp = sc_ps.tile([128, 512], F32)
                nc.tensor.matmul(sp[:], kt[:, t * 128:(t + 1) * 128],
                                 qsplit_sb[:, 512:1024], start=True, stop=True)
                nc.scalar.activation(es[t][:, 512:1024], sp[:],
                                     mybir.ActivationFunctionType.Exp,
                                     bias=shift_sb[:], scale=float(SCALE))
            return es

        def emit_attn_chunk(first_group, last_group, ci, size, es, vts):
            pA = oA_ps.tile([128, CA], F32)
            pB = oB_ps.tile([128, CB], F32)
            # very last chunk: preload acc into PSUM (DVE, early) and
            # accumulate on top with start=False, so the tail skips the
            # PSUM->SBUF flush and normalizes straight out of PSUM
            preload = last_group and ci == nchunks - 1
            if preload:
                nc.vector.tensor_copy(pA[:], acc[:, ci, 0:CA])
                nc.vector.tensor_copy(pB[:], acc[:, ci, CA:caug])
            for i in range(size):
                e = es[i][:, ci * 128:(ci + 1) * 128]
                mms = [(pA, 0, CA), (pB, CA, caug)]
                if last_group:
                    # B (denominator) last-matmul finishes first, so its
                    # flush + the reciprocal overlap the final A matmul
                    mms.reverse()
                for pt, c0, c1 in mms:
                    nc.tensor.matmul(pt[:], e, vts[i][:, c0:c1],
                                     start=(i == 0 and not preload),
                                     stop=(i == size - 1),
                                     skip_group_check=preload)
            if preload:
                rec = recp.tile([128, 1], F32)
                nc.vector.reciprocal(rec[:], pB[:, CB - 1:CB])
                ot = outp.tile([128, C], F16)
                nc.vector.tensor_scalar_mul(ot[:, 0:CA], pA[:], rec[:])
                nc.vector.tensor_scalar_mul(ot[:, CA:C], pB[:, 0:C - CA],
                                            rec[:])
                nc.sync.dma_start(out_d[ci * 128:(ci + 1) * 128, :], ot[:])
                return
            # flush B (with the denominator) first so the last chunk's
            # reciprocal overlaps the A flush
            if first_group:
                # fold in the host-computed zero-row contribution
                nc.vector.tensor_add(acc[:, ci, CA:caug], vz_sb[:, CA:caug],
                                     pB[:])
            else:
                nc.vector.tensor_add(acc[:, ci, CA:caug], acc[:, ci, CA:caug],
                                     pB[:])
            if last_group:
                rec = recp.tile([128, 1], F32)
                nc.vector.reciprocal(rec[:], acc[:, ci, C:caug])
            if first_group:
                nc.vector.tensor_add(acc[:, ci, 0:CA], vz_sb[:, 0:CA], pA[:])
            else:
                nc.vector.tensor_add(acc[:, ci, 0:CA], acc[:, ci, 0:CA], pA[:])
            if last_group:
                # normalize and store this chunk immediately (short tail)
                ot = outp.tile([128, C], F16)
                nc.vector.tensor_scalar_mul(ot[:], acc[:, ci, 0:C], rec[:])
                nc.sync.dma_start(out_d[ci * 128:(ci + 1) * 128, :], ot[:])

        # ---- software-pipelined main loop ----
        # group 0's k/v DMAs go out on the Activation HWDGE queue so their
        # ~600ns triggers issue in parallel with Sync's qsplit/shift (the
        # serial trigger chain otherwise gates the first matmul)
        ks_cur = emit_k(starts[0], sizes[0], eng=nc.scalar)
        v_cur = emit_v(starts[0], sizes[0], eng=nc.scalar)
        vz_sb = const.tile([128, caug], F32)
        nc.scalar.dma_start(vz_sb[:], vz_d[:])
        e_cur = emit_scores(ks_cur, range(sizes[0]), h_major=True)
        for gi in range(len(sizes)):
            last = gi + 1 >= len(sizes)
            if not last:
                ks_nxt = emit_k(starts[gi + 1], sizes[gi + 1])
                v_nxt = emit_v(starts[gi + 1], sizes[gi + 1])
                e_nxt = []
            # distribute next group's score matmuls across this group's
            # attn chunks to keep PE dense and ACT fed early
            for ci in range(nchunks):
                emit_attn_chunk(gi == 0, last, ci, sizes[gi], e_cur, v_cur)
                if not last:
                    nnx = sizes[gi + 1]
                    per = (nnx + nchunks - 1) // nchunks
                    ts = range(ci * per, min((ci + 1) * per, nnx))
                    e_nxt.extend(emit_scores(ks_nxt, ts))
            if not last:
                ks_cur, v_cur, e_cur = ks_nxt, v_nxt, e_nxt

    nc.finalize()
    return nc


def _split16(x):
    hi = x.astype(np.float16)
    lo = (x - hi.astype(np.float32)).astype(np.float16)
    return hi, lo


def _qsplit(qp):
    """[n, 3] projected queries -> [128, n] fp16, blocks [hi, lo, hi, lo]."""
    hi, lo = _split16(np.ascontiguousarray(qp.T.astype(np.float32)))
    m = np.zeros((128, qp.shape[0]), np.float16)
    for e in range(3):
        m[0 + e] = hi[e]
        m[32 + e] = lo[e]
        m[64 + e] = hi[e]
        m[96 + e] = lo[e]
    return m


def _ksplit(kp, nf_eff):
    """[nz, 3] projected keys -> [128, nf_eff] fp16, blocks
    [hi, hi, lo, lo], zero-padded beyond nz."""
    hi, lo = _split16(np.ascontiguousarray(kp.T.astype(np.float32)))
    m = np.zeros((128, nf_eff), np.float16)
    nz = kp.shape[0]
    for e in range(3):
        m[0 + e, :nz] = hi[e]
        m[32 + e, :nz] = hi[e]
        m[64 + e, :nz] = lo[e]
        m[96 + e, :nz] = lo[e]
    return m


def _host_prep(q, k, v, W1, b1, W2, b2):
    """Per-core input maps: tiny projections, fp16 splits, zero-row
    compaction, shift bound. O(N*D) host work only."""
    import ml_dtypes

    caug = C + 1
    qps, kps, nzs = [], [], []
    for b in range(B):
        qps.append(np.maximum(q[b].astype(np.float32) @ W1.T.astype(np.float32)
                              + b1.astype(np.float32), 0.0))
        kps.append(np.maximum(k[b].astype(np.float32) @ W2.T.astype(np.float32)
                              + b2.astype(np.float32), 0.0))
        nzs.append(np.flatnonzero((kps[b] > 0.0).any(axis=1)))
    nf_eff = max((len(nz) + 127) // 128 * 128 for nz in nzs)

    per_batch = {}
    for b in range(B):
        qp, kp, nz = qps[b], kps[b], nzs[b]
        nzmask = np.zeros(NF, dtype=bool)
        nzmask[nz] = True
        zcount = NF - len(nz)
        # cheap per-batch upper bound on max score -> exp(s - shift) <= 1
        bound = SCALE * float(qp.max(axis=0) @ kp.max(axis=0))
        w0 = float(np.exp(-bound))

        va = np.zeros((nf_eff, caug), np.float32)
        va[:len(nz), :C] = v[b][nz]
        va[:len(nz), C] = 1.0
        # [nf_eff, caug] -> [128, m_tiles, caug] with row(t*128+p) at [p, t]
        va = np.ascontiguousarray(
            va.reshape(nf_eff // 128, 128, caug).transpose(1, 0, 2))

        vzrow = np.empty(caug, np.float64)
        vzrow[:C] = v[b][~nzmask].astype(np.float64).sum(axis=0) * w0
        vzrow[C] = zcount * w0

        per_batch[b] = {
            "ksplit": _ksplit(kp[nz], nf_eff),
            "vaug": va.astype(ml_dtypes.bfloat16),
            "vz": np.ascontiguousarray(
                np.tile(vzrow.astype(np.float32), (128, 1))),
            "shift": np.full((128, 1), -bound, np.float32),
        }

    in_maps = []
    for core in range(8):
        b, h = core // 2, core % 2
        in_maps.append({
            "qsplit": _qsplit(qps[b][h * NQ:(h + 1) * NQ]),
            **per_batch[b],
        })
    return in_maps, nf_eff


_NC_CACHE = {}


def kernel(q, k, v, W1, b1, W2, b2, _trace=False):
    q, k, v = np.asarray(q), np.asarray(k), np.asarray(v)
    W1, b1 = np.asarray(W1), np.asarray(b1)
    W2, b2 = np.asarray(W2), np.asarray(b2)

    in_maps, nf_eff = _host_prep(q, k, v, W1, b1, W2, b2)
    m_tiles = nf_eff // 128
    if m_tiles not in _NC_CACHE:
        _NC_CACHE[m_tiles] = build_nc(m_tiles)
    nc = _NC_CACHE[m_tiles]

    res = run_bass_kernel_spmd(nc, in_maps, list(range(8)), trace=_trace)

    out = np.empty((B, NQ_FULL, C), np.float32)
    for core in range(8):
        b, h = core // 2, core % 2
        out[b, h * NQ:(h + 1) * NQ, :] = np.asarray(
            res.results[core]["out"]).astype(np.float32)
    if _trace:
        return out, res
    return out
